# revision 7
# baseline (speedup 1.0000x reference)
"""Bahdanau additive attention on 8 TRN2 NeuronCores.

Problem (hardcoded shapes):
  B=8, Ld=128, Le=512, n_enc=n_dec=512, n_att=256
  pe = h_e @ W_en.T + b_en          # (B, Le, n_att)
  pd = h_d @ W_de.T                 # (B, Ld, n_att)
  scores[b,d,e] = sum_n W_att[n] * tanh(pd[b,d,n] + pe[b,e,n])  (+ b_att, dropped:
                  softmax is shift-invariant)
  p = softmax(scores, axis=e) * mask;  p /= (sum_e p + 1e-8)

Sharding: data-parallel over batch B across the 8 cores (one batch element
per core, no collectives).

Per-core pipeline (ScalarE-bound: 16.7M tanh evaluations at 1 elem/lane/cyc):
  - VectorE (+ a slice on GpSimd): X = pe_T + pd_T[:,d] broadcast adds
    (bf16 tensor_scalar), PSUM window drains, softmax sums/renorm.
  - ScalarE: one big tanh per 16-decoder-step window (amortizes the ~400-cycle
    per-call overhead), exp for softmax, prologue PSUM->SBUF copies.
  - TensorE: projections (bf16); n-reduction with W_att chunk as the 1-column
    stationary operand and the tanh tile as the 512-wide moving operand
    (moving path streams at 2.4 GHz vs 1.2 for LDWEIGHTS, and fp32 matmul
    would run half-rate in LOW_HIGH mode). Scores rows land at PSUM
    partitions {0,32,64,96} via column tile_position, 4 decoder steps per
    bank, 4 banks = one window tile; a start=True zero-matmul per bank
    pre-sets every element's has_written bit so all real matmuls are
    order-independent accumulates.
  - Scores rows sit scattered at partitions {0,32,64,96}: one wide DVE drain
    per window, then partition-remap via DRAM bounce (DMA with strided
    DRAM-side access pattern; strided SBUF partition APs don't work).
Host-side prep is layout only: batch slicing, transposes so contraction dims
land on partitions, and bf16 casts of the matmul inputs.
"""

import numpy as np

B, Ld, Le = 8, 128, 512
N_ENC = N_DEC = 512
N_ATT = 256
KC = 4  # contraction chunks of 128 over n_enc/n_dec
NC_CHUNKS = 2  # n_att = 2 chunks of 128
DW = 16  # decoder steps per tanh window (one big ACT call each)
SUPER = 32  # decoder steps per remap super-group (2 windows)
GP_OFF = 4  # broadcast-adds per window offloaded DVE -> GpSimd

_CACHE = {}


def _build_nc():
    import concourse.mybir as mybir
    import concourse.tile as tile
    from concourse import bacc
    from concourse.bass import ts

    f32 = mybir.dt.float32
    bf16 = mybir.dt.bfloat16
    AF = mybir.ActivationFunctionType
    ALU = mybir.AluOpType

    nc = bacc.Bacc("TRN2", target_bir_lowering=False, debug=False, num_devices=B)

    h_eT = nc.declare_dram_parameter("h_eT", [N_ENC, Le], bf16, isOutput=False)
    h_dT = nc.declare_dram_parameter("h_dT", [N_DEC, Ld], bf16, isOutput=False)
    w_enT = nc.declare_dram_parameter("W_enT", [N_ENC, N_ATT], bf16, isOutput=False)
    w_deT = nc.declare_dram_parameter("W_deT", [N_DEC, N_ATT], bf16, isOutput=False)
    w_att = nc.declare_dram_parameter("W_att2", [128, NC_CHUNKS], bf16, isOutput=False)
    b_en = nc.declare_dram_parameter("b_en2", [128, NC_CHUNKS], f32, isOutput=False)
    mask = nc.declare_dram_parameter("mask", [1, Le], f32, isOutput=False)
    out = nc.declare_dram_parameter("out", [Ld, Le], f32, isOutput=True)

    with tile.TileContext(nc) as tc:
        with (
            tc.tile_pool(name="weights", bufs=1) as wpool,
            tc.tile_pool(name="proj", bufs=1) as projpool,
            tc.tile_pool(name="xw", bufs=3) as xpool,
            tc.tile_pool(name="stage", bufs=2) as spool,
            tc.tile_pool(name="soft", bufs=1) as softpool,
            tc.tile_pool(name="dram", bufs=1, space="DRAM") as dram_pool,
        ):
            # ---- loads, critical-path first, split across both HWDGE queues ----
            wenT_sb = wpool.tile([128, KC, N_ATT], bf16)
            nc.sync.dma_start(wenT_sb[:], w_enT[:].rearrange("(c p) n -> p c n", p=128))
            heT_sb = wpool.tile([128, KC, Le], bf16)
            nc.sync.dma_start(heT_sb[:], h_eT[:].rearrange("(c p) e -> p c e", p=128))
            wdeT_sb = wpool.tile([128, KC, N_ATT], bf16)
            nc.scalar.dma_start(wdeT_sb[:], w_deT[:].rearrange("(c p) n -> p c n", p=128))
            hdT_sb = wpool.tile([128, KC, Ld], bf16)
            nc.scalar.dma_start(hdT_sb[:], h_dT[:].rearrange("(c p) d -> p c d", p=128))
            watt_sb = wpool.tile([128, NC_CHUNKS], bf16)
            nc.scalar.dma_start(watt_sb[:], w_att[:])
            ben_sb = wpool.tile([128, NC_CHUNKS], f32)
            nc.scalar.dma_start(ben_sb[:], b_en[:])
            mask_sb = wpool.tile([1, Le], f32)
            nc.scalar.dma_start(mask_sb[:], mask[:])
            ones_sb = wpool.tile([1, 128], f32)
            nc.vector.memset(ones_sb[:], 1.0)
            zeros_sb = wpool.tile([1, Le], bf16)
            nc.vector.memset(zeros_sb[:], 0.0)

            # ---- prologue: projections + mask broadcast (own PSUM scope) ----
            pe_bf = projpool.tile([128, NC_CHUNKS, Le], bf16)
            pd_sb = projpool.tile([128, NC_CHUNKS, Ld], f32)
            mask_b = softpool.tile([128, Le], f32)
            with tc.tile_pool(name="ps_proj", bufs=1, space="PSUM") as ps_proj:
                # pe_T[n, e] (+ b_en): bias fused into the ACT PSUM->SBUF copy
                for m in range(NC_CHUNKS):
                    ps = ps_proj.tile([128, Le], f32, tag="ps_pe")
                    for k in range(KC):
                        nc.tensor.matmul(
                            ps[:],
                            lhsT=wenT_sb[:, k, ts(m, 128)],
                            rhs=heT_sb[:, k, :],
                            start=(k == 0),
                            stop=(k == KC - 1),
                        )
                    nc.scalar.activation(pe_bf[:, m, :], ps[:], AF.Identity,
                                         bias=ben_sb[:, m : m + 1])

                for m in range(NC_CHUNKS):
                    ps = ps_proj.tile([128, Ld], f32, tag="ps_pd")
                    for k in range(KC):
                        nc.tensor.matmul(
                            ps[:],
                            lhsT=wdeT_sb[:, k, ts(m, 128)],
                            rhs=hdT_sb[:, k, :],
                            start=(k == 0),
                            stop=(k == KC - 1),
                        )
                    nc.scalar.copy(pd_sb[:, m, :], ps[:])

                ps_mask = ps_proj.tile([128, Le], f32, tag="ps_mask")
                nc.tensor.matmul(ps_mask[:], lhsT=ones_sb[:], rhs=mask_sb[:],
                                 start=True, stop=True)
                nc.scalar.copy(mask_b[:], ps_mask[:])

            # ---- main: per 16-d window: adds -> one big tanh -> 16 MMs -> drain ----
            scores_stage = dram_pool.tile([Ld, Le], f32)
            with tc.tile_pool(name="ps_w", bufs=2, space="PSUM") as ps_w:
                n_win = Ld // DW
                stage_sb = None
                for w in range(n_win):
                    if w % 2 == 0:
                        stage_sb = spool.tile([128, SUPER // 4, Le], f32, tag="S")
                    pw = ps_w.tile([128, 4, Le], f32, tag="pw")  # 4 banks
                    for q in range(4):
                        nc.tensor.matmul(pw[:, q, :], lhsT=zeros_sb[:, 0:128],
                                         rhs=zeros_sb[:], start=True, stop=False)
                    for c in range(NC_CHUNKS):
                        x = xpool.tile([128, DW, Le], bf16, tag="X")
                        for i in range(DW):
                            d = w * DW + i
                            eng = nc.gpsimd if i < GP_OFF else nc.vector
                            eng.tensor_scalar(
                                x[:, i, :], pe_bf[:, c, :],
                                pd_sb[:, c, d : d + 1], None, op0=ALU.add)
                        nc.scalar.activation(x[:], x[:], AF.Tanh)
                        for i in range(DW):
                            q, j = i // 4, i % 4
                            nc.tensor.matmul(
                                pw[32 * j : 32 * j + 1, q, :],
                                lhsT=watt_sb[:, c : c + 1],
                                rhs=x[:, i, :],
                                start=False,
                                stop=(c == NC_CHUNKS - 1),
                                tile_position=(0, 32 * j),
                            )
                    # wide drain of the 4 completed banks
                    nc.vector.tensor_copy(
                        stage_sb[:, 4 * (w % 2) : 4 * (w % 2) + 4, :], pw[:])
                    if w % 2 == 1:
                        # partition remap via DRAM-side strided access pattern:
                        # stage_sb[32j, t, :] holds scores row d = 32s + 4t + j
                        s = w // 2
                        for j in range(4):
                            nc.sync.dma_start(
                                scores_stage[32 * s + j : 32 * s + j + 29 : 4, :],
                                stage_sb[32 * j : 32 * j + 1, :, :],
                            )

            # ---- softmax over e, mask, renormalise (all SBUF) ----
            # out = E*mask / (sum(E*mask) + EPS*Z),  E = exp(s), Z = sum(E)
            # (identical to ref: softmax, *mask, /(sum+EPS); b_att dropped —
            #  softmax is shift-invariant)
            scores_sb = softpool.tile([128, Le], f32)
            nc.sync.dma_start(scores_sb[:], scores_stage[:])
            ex = softpool.tile([128, Le], f32)
            nc.scalar.activation(ex[:], scores_sb[:], AF.Exp)
            z = softpool.tile([128, 1], f32)
            nc.vector.tensor_reduce(z[:], ex[:], axis=mybir.AxisListType.X,
                                    op=ALU.add)
            em = softpool.tile([128, Le], f32)
            nc.vector.tensor_mul(em[:], ex[:], mask_b[:])
            s2 = softpool.tile([128, 1], f32)
            nc.vector.tensor_reduce(s2[:], em[:], axis=mybir.AxisListType.X,
                                    op=ALU.add)
            den = softpool.tile([128, 1], f32)
            nc.vector.tensor_scalar(den[:], z[:], 1e-8, None, op0=ALU.mult)
            nc.vector.tensor_add(den[:], den[:], s2[:])
            rec = softpool.tile([128, 1], f32)
            nc.vector.reciprocal(rec[:], den[:])
            res = softpool.tile([128, Le], f32)
            nc.vector.tensor_scalar(res[:], em[:], rec[:], None, op0=ALU.mult)
            nc.sync.dma_start(out[:], res[:])

    nc.compile()
    return nc


def _in_maps(h_e, h_d, mask, W_en, b_en, W_de, W_att):
    import ml_dtypes

    f = np.float32
    bf = ml_dtypes.bfloat16
    w_enT = np.ascontiguousarray(W_en.T.astype(bf))
    w_deT = np.ascontiguousarray(W_de.T.astype(bf))
    w_att2 = np.ascontiguousarray(W_att.reshape(NC_CHUNKS, 128).T.astype(bf))
    b_en2 = np.ascontiguousarray(b_en.reshape(NC_CHUNKS, 128).T, dtype=f)
    maps = []
    for b in range(B):
        maps.append({
            "h_eT": np.ascontiguousarray(h_e[b].T.astype(bf)),
            "h_dT": np.ascontiguousarray(h_d[b].T.astype(bf)),
            "W_enT": w_enT,
            "W_deT": w_deT,
            "W_att2": w_att2,
            "b_en2": b_en2,
            "mask": np.ascontiguousarray(mask[b : b + 1, :], dtype=f),
        })
    return maps


def run(h_e, h_d, mask, W_en, b_en, W_de, W_att, b_att=None, trace=False,
        **trace_kwargs):
    from concourse.bass_utils import run_bass_kernel_spmd

    if "nc" not in _CACHE:
        _CACHE["nc"] = _build_nc()
    nc = _CACHE["nc"]
    maps = _in_maps(np.asarray(h_e), np.asarray(h_d), np.asarray(mask),
                    np.asarray(W_en), np.asarray(b_en), np.asarray(W_de),
                    np.asarray(W_att))
    res = run_bass_kernel_spmd(nc, maps, core_ids=list(range(B)), trace=trace,
                               **trace_kwargs)
    p = np.stack([np.asarray(res.results[b]["out"]) for b in range(B)], axis=0)
    return p.astype(np.float32), res


def kernel(h_e, h_d, mask, W_en, b_en, W_de, W_att, b_att):
    p, _ = run(h_e, h_d, mask, W_en, b_en, W_de, W_att, b_att)
    return p


# revision 8
# speedup vs baseline: 3.7109x; 3.7109x over previous
"""Bahdanau additive attention on 8 TRN2 NeuronCores.

Problem (hardcoded shapes):
  B=8, Ld=128, Le=512, n_enc=n_dec=512, n_att=256
  pe = h_e @ W_en.T + b_en          # (B, Le, n_att)
  pd = h_d @ W_de.T                 # (B, Ld, n_att)
  scores[b,d,e] = sum_n W_att[n] * tanh(pd[b,d,n] + pe[b,e,n])  (+ b_att, dropped:
                  softmax is shift-invariant)
  p = softmax(scores, axis=e) * mask;  p /= (sum_e p + 1e-8)

Sharding: data-parallel over batch B across the 8 cores (one batch element
per core, no collectives).

Per-core pipeline (ScalarE-bound: 16.7M tanh evaluations at 1 elem/lane/cyc):
  - VectorE (+ a slice on GpSimd): X = pe_T + pd_T[:,d] broadcast adds
    (bf16 tensor_scalar), PSUM window drains, softmax sums/renorm.
  - ScalarE: one big tanh per 16-decoder-step window (amortizes the ~400-cycle
    per-call overhead), exp for softmax, prologue PSUM->SBUF copies.
  - TensorE: projections (bf16); n-reduction with W_att chunk as the 1-column
    stationary operand and the tanh tile as the 512-wide moving operand
    (moving path streams at 2.4 GHz vs 1.2 for LDWEIGHTS, and fp32 matmul
    would run half-rate in LOW_HIGH mode). Scores rows land at PSUM
    partitions {0,32,64,96} via column tile_position, 4 decoder steps per
    bank, 4 banks = one window tile; a start=True zero-matmul per bank
    pre-sets every element's has_written bit so all real matmuls are
    order-independent accumulates.
  - Scores rows sit scattered at partitions {0,32,64,96}: one wide DVE drain
    per window, then partition-remap via DRAM bounce (DMA with strided
    DRAM-side access pattern; strided SBUF partition APs don't work).
Host-side prep is layout only: batch slicing, transposes so contraction dims
land on partitions, and bf16 casts of the matmul inputs.
"""

import numpy as np

B, Ld, Le = 8, 128, 512
N_ENC = N_DEC = 512
N_ATT = 256
KC = 4  # contraction chunks of 128 over n_enc/n_dec
NC_CHUNKS = 2  # n_att = 2 chunks of 128
DW = 16  # decoder steps per tanh window (one big ACT call each)
SUPER = 32  # decoder steps per remap super-group (2 windows)
GP_OFF = 0  # broadcast-adds per window offloaded DVE -> GpSimd
# (GpSimd tensor_scalar measured ~7.4us per [128,512] call AND its SBUF port
#  lock drags concurrent DVE tensor_scalar from ~350ns to ~2.6us — never use.)

_CACHE = {}


def _build_nc():
    import concourse.mybir as mybir
    import concourse.tile as tile
    from concourse import bacc
    from concourse.bass import ts

    f32 = mybir.dt.float32
    bf16 = mybir.dt.bfloat16
    AF = mybir.ActivationFunctionType
    ALU = mybir.AluOpType

    nc = bacc.Bacc("TRN2", target_bir_lowering=False, debug=False, num_devices=B)

    h_eT = nc.declare_dram_parameter("h_eT", [N_ENC, Le], bf16, isOutput=False)
    h_dT = nc.declare_dram_parameter("h_dT", [N_DEC, Ld], bf16, isOutput=False)
    w_enT = nc.declare_dram_parameter("W_enT", [N_ENC, N_ATT], bf16, isOutput=False)
    w_deT = nc.declare_dram_parameter("W_deT", [N_DEC, N_ATT], bf16, isOutput=False)
    w_att = nc.declare_dram_parameter("W_att2", [128, NC_CHUNKS], bf16, isOutput=False)
    b_en = nc.declare_dram_parameter("b_en2", [128, NC_CHUNKS], f32, isOutput=False)
    mask = nc.declare_dram_parameter("mask", [1, Le], f32, isOutput=False)
    out = nc.declare_dram_parameter("out", [Ld, Le], f32, isOutput=True)

    with tile.TileContext(nc) as tc:
        with (
            tc.tile_pool(name="weights", bufs=1) as wpool,
            tc.tile_pool(name="proj", bufs=1) as projpool,
            tc.tile_pool(name="xw", bufs=3) as xpool,
            tc.tile_pool(name="stage", bufs=2) as spool,
            tc.tile_pool(name="soft", bufs=1) as softpool,
            tc.tile_pool(name="dram", bufs=1, space="DRAM") as dram_pool,
        ):
            # ---- loads, critical-path first, split across both HWDGE queues ----
            wenT_sb = wpool.tile([128, KC, N_ATT], bf16)
            nc.sync.dma_start(wenT_sb[:], w_enT[:].rearrange("(c p) n -> p c n", p=128))
            heT_sb = wpool.tile([128, KC, Le], bf16)
            nc.sync.dma_start(heT_sb[:], h_eT[:].rearrange("(c p) e -> p c e", p=128))
            wdeT_sb = wpool.tile([128, KC, N_ATT], bf16)
            nc.scalar.dma_start(wdeT_sb[:], w_deT[:].rearrange("(c p) n -> p c n", p=128))
            hdT_sb = wpool.tile([128, KC, Ld], bf16)
            nc.scalar.dma_start(hdT_sb[:], h_dT[:].rearrange("(c p) d -> p c d", p=128))
            watt_sb = wpool.tile([128, NC_CHUNKS], bf16)
            nc.scalar.dma_start(watt_sb[:], w_att[:])
            ben_sb = wpool.tile([128, NC_CHUNKS], f32)
            nc.scalar.dma_start(ben_sb[:], b_en[:])
            mask_sb = wpool.tile([1, Le], f32)
            nc.scalar.dma_start(mask_sb[:], mask[:])
            ones_sb = wpool.tile([1, 128], f32)
            nc.vector.memset(ones_sb[:], 1.0)
            zeros_sb = wpool.tile([1, Le], bf16)
            nc.vector.memset(zeros_sb[:], 0.0)

            # ---- prologue: projections + mask broadcast (own PSUM scope) ----
            pe_bf = projpool.tile([128, NC_CHUNKS, Le], bf16)
            pd_sb = projpool.tile([128, NC_CHUNKS, Ld], f32)
            mask_b = softpool.tile([128, Le], f32)
            with tc.tile_pool(name="ps_proj", bufs=1, space="PSUM") as ps_proj:
                # pe_T[n, e] (+ b_en): bias fused into the ACT PSUM->SBUF copy
                for m in range(NC_CHUNKS):
                    ps = ps_proj.tile([128, Le], f32, tag="ps_pe")
                    for k in range(KC):
                        nc.tensor.matmul(
                            ps[:],
                            lhsT=wenT_sb[:, k, ts(m, 128)],
                            rhs=heT_sb[:, k, :],
                            start=(k == 0),
                            stop=(k == KC - 1),
                        )
                    nc.scalar.activation(pe_bf[:, m, :], ps[:], AF.Identity,
                                         bias=ben_sb[:, m : m + 1])

                for m in range(NC_CHUNKS):
                    ps = ps_proj.tile([128, Ld], f32, tag="ps_pd")
                    for k in range(KC):
                        nc.tensor.matmul(
                            ps[:],
                            lhsT=wdeT_sb[:, k, ts(m, 128)],
                            rhs=hdT_sb[:, k, :],
                            start=(k == 0),
                            stop=(k == KC - 1),
                        )
                    nc.scalar.copy(pd_sb[:, m, :], ps[:])

                ps_mask = ps_proj.tile([128, Le], f32, tag="ps_mask")
                nc.tensor.matmul(ps_mask[:], lhsT=ones_sb[:], rhs=mask_sb[:],
                                 start=True, stop=True)
                nc.scalar.copy(mask_b[:], ps_mask[:])

            # ---- main: per 16-d window: adds -> one big tanh -> 16 MMs -> drain ----
            scores_stage = dram_pool.tile([Ld, Le], f32)
            with tc.tile_pool(name="ps_w", bufs=2, space="PSUM") as ps_w:
                n_win = Ld // DW
                stage_sb = None
                for w in range(n_win):
                    if w % 2 == 0:
                        stage_sb = spool.tile([128, SUPER // 4, Le], f32, tag="S")
                    pw = ps_w.tile([128, 4, Le], f32, tag="pw")  # 4 banks
                    for q in range(4):
                        nc.tensor.matmul(pw[:, q, :], lhsT=zeros_sb[:, 0:128],
                                         rhs=zeros_sb[:], start=True, stop=False)
                    for c in range(NC_CHUNKS):
                        x = xpool.tile([128, DW, Le], bf16, tag="X")
                        for i in range(DW):
                            d = w * DW + i
                            eng = nc.gpsimd if i < GP_OFF else nc.vector
                            eng.tensor_scalar(
                                x[:, i, :], pe_bf[:, c, :],
                                pd_sb[:, c, d : d + 1], None, op0=ALU.add)
                        nc.scalar.activation(x[:], x[:], AF.Tanh)
                        for i in range(DW):
                            q, j = i // 4, i % 4
                            nc.tensor.matmul(
                                pw[32 * j : 32 * j + 1, q, :],
                                lhsT=watt_sb[:, c : c + 1],
                                rhs=x[:, i, :],
                                start=False,
                                stop=(c == NC_CHUNKS - 1),
                                tile_position=(0, 32 * j),
                            )
                    # wide drain of the 4 completed banks
                    nc.vector.tensor_copy(
                        stage_sb[:, 4 * (w % 2) : 4 * (w % 2) + 4, :], pw[:])
                    if w % 2 == 1:
                        # partition remap via DRAM-side strided access pattern:
                        # stage_sb[32j, t, :] holds scores row d = 32s + 4t + j
                        s = w // 2
                        for j in range(4):
                            nc.sync.dma_start(
                                scores_stage[32 * s + j : 32 * s + j + 29 : 4, :],
                                stage_sb[32 * j : 32 * j + 1, :, :],
                            )

            # ---- softmax over e, mask, renormalise (all SBUF) ----
            # out = E*mask / (sum(E*mask) + EPS*Z),  E = exp(s), Z = sum(E)
            # (identical to ref: softmax, *mask, /(sum+EPS); b_att dropped —
            #  softmax is shift-invariant)
            scores_sb = softpool.tile([128, Le], f32)
            nc.sync.dma_start(scores_sb[:], scores_stage[:])
            ex = softpool.tile([128, Le], f32)
            nc.scalar.activation(ex[:], scores_sb[:], AF.Exp)
            z = softpool.tile([128, 1], f32)
            nc.vector.tensor_reduce(z[:], ex[:], axis=mybir.AxisListType.X,
                                    op=ALU.add)
            em = softpool.tile([128, Le], f32)
            nc.vector.tensor_mul(em[:], ex[:], mask_b[:])
            s2 = softpool.tile([128, 1], f32)
            nc.vector.tensor_reduce(s2[:], em[:], axis=mybir.AxisListType.X,
                                    op=ALU.add)
            den = softpool.tile([128, 1], f32)
            nc.vector.tensor_scalar(den[:], z[:], 1e-8, None, op0=ALU.mult)
            nc.vector.tensor_add(den[:], den[:], s2[:])
            rec = softpool.tile([128, 1], f32)
            nc.vector.reciprocal(rec[:], den[:])
            res = softpool.tile([128, Le], f32)
            nc.vector.tensor_scalar(res[:], em[:], rec[:], None, op0=ALU.mult)
            nc.sync.dma_start(out[:], res[:])

    nc.compile()
    return nc


def _in_maps(h_e, h_d, mask, W_en, b_en, W_de, W_att):
    import ml_dtypes

    f = np.float32
    bf = ml_dtypes.bfloat16
    w_enT = np.ascontiguousarray(W_en.T.astype(bf))
    w_deT = np.ascontiguousarray(W_de.T.astype(bf))
    w_att2 = np.ascontiguousarray(W_att.reshape(NC_CHUNKS, 128).T.astype(bf))
    b_en2 = np.ascontiguousarray(b_en.reshape(NC_CHUNKS, 128).T, dtype=f)
    maps = []
    for b in range(B):
        maps.append({
            "h_eT": np.ascontiguousarray(h_e[b].T.astype(bf)),
            "h_dT": np.ascontiguousarray(h_d[b].T.astype(bf)),
            "W_enT": w_enT,
            "W_deT": w_deT,
            "W_att2": w_att2,
            "b_en2": b_en2,
            "mask": np.ascontiguousarray(mask[b : b + 1, :], dtype=f),
        })
    return maps


def run(h_e, h_d, mask, W_en, b_en, W_de, W_att, b_att=None, trace=False,
        **trace_kwargs):
    from concourse.bass_utils import run_bass_kernel_spmd

    if "nc" not in _CACHE:
        _CACHE["nc"] = _build_nc()
    nc = _CACHE["nc"]
    maps = _in_maps(np.asarray(h_e), np.asarray(h_d), np.asarray(mask),
                    np.asarray(W_en), np.asarray(b_en), np.asarray(W_de),
                    np.asarray(W_att))
    res = run_bass_kernel_spmd(nc, maps, core_ids=list(range(B)), trace=trace,
                               **trace_kwargs)
    p = np.stack([np.asarray(res.results[b]["out"]) for b in range(B)], axis=0)
    return p.astype(np.float32), res


def kernel(h_e, h_d, mask, W_en, b_en, W_de, W_att, b_att):
    p, _ = run(h_e, h_d, mask, W_en, b_en, W_de, W_att, b_att)
    return p
